# revision 18
# baseline (speedup 1.0000x reference)
"""FlowNetC correlation (kernel_size=1, max_disp=20, stride2=2) on 8 Trainium2 cores.

Problem: inputs input1, input2 of shape [8, 256, 64, 96] fp32; output
[8, 441, 64, 96] fp32 with
  out[b, i*21+j, y, x] = (1/256) * sum_c in1[b,c,y,x] * in2[b,c,y+2i-20,x+2j-20]
(zero where the in2 index is out of range).

Sharding: data-parallel over batch — core b handles batch element b.

Per-core device strategy: tile (y, x) into 48 parity-separated blocks of
BY*BX = 8*16 = 128 pixels.  The TensorEngine computes, for each block,
P[m, (r,u)] = sum_c in1[c, y_m, x_m] * in2[c, r, u] over the block's
displacement window (a [128, <=26x36] fp16 matmul per 128-channel chunk,
accumulated fp32 in PSUM over the two channel chunks).  The device writes
these banded products to DRAM contiguously (fp16, four same-geometry blocks
per transfer); the host scatters the valid (m, r, u) entries into the final
output (the band->output map is a fixed sparse permutation, precomputed
once).  Inputs stream in piecewise so matmuls overlap the loads; the kernel
is DMA-bound at ~14 MB/core (cost-model estimate ~47 us/core).
"""

import numpy as np

C, H, W = 256, 64, 96
D = 21
PADV = 20
B = 8
N_CORES = 8
BY, BX = 8, 16
NBLK = 48
QSTRIDE = 512  # psum bank size in fp32 elements
OUT_FREE = 2 * QSTRIDE


def _block_geometry():
    blocks = []
    for y0 in (0, 16, 32, 48):
        for x0 in (0, 32, 64):
            for py in (0, 1):
                for px in (0, 1):
                    ys = [y0 + py + 2 * b for b in range(BY)]
                    xs = [x0 + px + 2 * a for a in range(BX)]
                    r_lo = ys[0] - PADV
                    while r_lo < 0:
                        r_lo += 2
                    r_hi = min(ys[-1] + PADV, H - 1)
                    rs = list(range(r_lo, r_hi + 1, 2))
                    u_lo = xs[0] - PADV
                    while u_lo < 0:
                        u_lo += 2
                    u_hi = min(xs[-1] + PADV, W - 1)
                    us = list(range(u_lo, u_hi + 1, 2))
                    nu = len(us)
                    nmax = QSTRIDE // nu
                    nr = len(rs)
                    if nr <= nmax:
                        chunks = [(0, nr)]
                    else:
                        # split the r-range evenly across the two PSUM banks
                        n0 = (nr + 1) // 2
                        chunks = [(0, n0), (n0, nr - n0)]
                        assert n0 <= nmax
                    assert all(n * nu >= 256 for _, n in chunks), (nr, nu, chunks)
                    blocks.append(dict(ys=ys, xs=xs, rs=rs, us=us, chunks=chunks))
    assert len(blocks) == NBLK
    return blocks


_BLOCKS = _block_geometry()
_GATHER = None
_PROGRAM = None

# per-block pixel coordinates: YM[blk, m], XM[blk, m] with m = b*BX + a
_YM = np.array([np.repeat(g["ys"], BX) for g in _BLOCKS])
_XM = np.array([np.tile(g["xs"], BY) for g in _BLOCKS])


def _build_gather():
    """Flat indices such that O.flat[dst] = R.flat[src] for one core."""
    dst_list, src_list = [], []
    for blk, g in enumerate(_BLOCKS):
        ys = np.asarray(g["ys"])
        xs = np.asarray(g["xs"])
        rs = np.asarray(g["rs"])
        us = np.asarray(g["us"])
        nu = len(us)
        y_m = np.repeat(ys, BX)
        x_m = np.tile(xs, BY)
        for q, (off, n) in enumerate(g["chunks"]):
            r_q = rs[off : off + n]
            m_idx = np.arange(128)[:, None, None]
            ir = np.arange(n)[None, :, None]
            iu = np.arange(nu)[None, None, :]
            i = (r_q[None, :, None] - y_m[:, None, None] + PADV) // 2
            j = (us[None, None, :] - x_m[:, None, None] + PADV) // 2
            valid = (i >= 0) & (i < D) & (j >= 0) & (j < D)
            d = i * D + j
            dst = (d * H + y_m[:, None, None]) * W + x_m[:, None, None]
            src = ((blk // 4) * 128 + m_idx) * 4 * OUT_FREE + (
                blk % 4
            ) * OUT_FREE + (off + ir) * nu + iu
            bcast = np.broadcast_arrays(dst, src, valid)
            dst_list.append(bcast[0][valid])
            src_list.append(bcast[1][valid])
    return np.concatenate(dst_list), np.concatenate(src_list)


def _gather_indices():
    global _GATHER
    if _GATHER is None:
        _GATHER = _build_gather()
    return _GATHER


def _build_program():
    from contextlib import ExitStack

    import concourse.bacc as bacc
    import concourse.mybir as mybir
    import concourse.tile as tile

    # fp16 inputs: full-rate PE streaming, half the DMA bytes of fp32, and
    # ~3e-4 relative error on this problem (inputs are unit-scale randn).
    in_dt = mybir.dt.float16
    out_dt = mybir.dt.float16

    nc = bacc.Bacc("TRN2", target_bir_lowering=False, debug=False)
    # in1 is pre-packed on the host: [p, kc, blk, m] = in1[kc*128+p, YM[blk,m], XM[blk,m]]
    in1_d = nc.dram_tensor("in1", [128, 2, NBLK, 128], in_dt, kind="ExternalInput")
    in2_d = nc.dram_tensor("in2", [128, 2, H, W], in_dt, kind="ExternalInput")
    out_d = nc.dram_tensor(
        "out", [NBLK // 4, 128, 4, OUT_FREE], out_dt, kind="ExternalOutput"
    )

    with ExitStack() as ctx:
        tc = ctx.enter_context(tile.TileContext(nc))
        inp_pool = ctx.enter_context(tc.tile_pool(name="inp", bufs=1))
        psum_pool = ctx.enter_context(tc.tile_pool(name="psum", bufs=4, space="PSUM"))
        out_pool = ctx.enter_context(tc.tile_pool(name="outp", bufs=6))

        in1_s = inp_pool.tile([128, 2, NBLK, 128], in_dt)
        in2_s = inp_pool.tile([128, 2, H, W], in_dt)

        # Piecewise input loads so matmuls start while later pieces are in
        # flight (Tile emits region-granular waits).  Blocks of y0-group gy
        # need in1 blocks [12*gy, 12*gy+12) and in2 rows [16*gy-20, 16*gy+35].
        def load_in1_group(gy, kc):
            nc.scalar.dma_start(
                in1_s[:, kc, 12 * gy : 12 * (gy + 1), :],
                in1_d[:, kc, 12 * gy : 12 * (gy + 1), :],
            )

        def load_in2_rows(gr, kc):
            nc.scalar.dma_start(
                in2_s[:, kc, 16 * gr : 16 * (gr + 1), :],
                in2_d[:, kc, 16 * gr : 16 * (gr + 1), :],
            )

        # all pieces enqueue up front (no deps); emission order sets priority
        # so early y0-groups land first and matmuls start ~7 us in
        for kc in (0, 1):
            load_in1_group(0, kc)
            load_in2_rows(0, kc)
            load_in2_rows(1, kc)
            load_in2_rows(2, kc)
        for kc in (0, 1):
            load_in2_rows(3, kc)
            load_in1_group(1, kc)
        for kc in (0, 1):
            load_in1_group(2, kc)
            load_in1_group(3, kc)

        for blk, g in enumerate(_BLOCKS):
            ys, xs, rs, us = g["ys"], g["xs"], g["rs"], g["us"]
            nu = len(us)
            u0 = us[0]
            pt = psum_pool.tile([128, 2, QSTRIDE], mybir.dt.float32, tag="pt")
            if blk % 4 == 0:
                st = out_pool.tile([128, 4, OUT_FREE], out_dt, tag="st")
            for kc in (0, 1):
                lhsT = in1_s[:, kc, blk, :]
                for q, (off, n) in enumerate(g["chunks"]):
                    r0 = rs[off]
                    rhs = in2_s[
                        :, kc, r0 : r0 + 2 * n - 1 : 2, u0 : u0 + 2 * nu - 1 : 2
                    ]
                    nc.tensor.matmul(
                        pt[:, q, : n * nu],
                        lhsT,
                        rhs,
                        start=(kc == 0),
                        stop=(kc == 1),
                    )
            copy = nc.vector.tensor_copy if blk % 2 == 0 else nc.scalar.copy
            for q, (off, n) in enumerate(g["chunks"]):
                copy(
                    st[:, blk % 4, off * nu : (off + n) * nu], pt[:, q, : n * nu]
                )
            if blk % 4 == 3:
                # quad blocks (py/px combos of one (y0, x0)) share geometry
                ntot = len(rs) * nu
                nc.sync.dma_start(out_d[blk // 4, :, :, :ntot], st[:, :, :ntot])

    nc.compile()
    return nc


def _program():
    global _PROGRAM
    if _PROGRAM is None:
        _PROGRAM = _build_program()
    return _PROGRAM


def _prep_in1(x):
    # [256, 64, 96] -> [128, 2, NBLK, 128]: blocks of in1 pixels packed contiguously
    x2 = x.reshape(2, 128, H, W)
    g = x2[:, :, _YM, _XM]  # [2, 128, NBLK, 128]
    return np.ascontiguousarray(g.transpose(1, 0, 2, 3), dtype=np.float16)


def _prep_in2(x):
    # [256, 64, 96] -> [128, 2, 64, 96] with c = kc*128 + p laid out [p, kc, y, x]
    return np.ascontiguousarray(
        x.reshape(2, 128, H, W).transpose(1, 0, 2, 3), dtype=np.float16
    )


def make_in_maps(input1, input2):
    in1 = np.asarray(input1, dtype=np.float32)
    in2 = np.asarray(input2, dtype=np.float32)
    return [
        {"in1": _prep_in1(in1[b]), "in2": _prep_in2(in2[b])} for b in range(B)
    ]


def extract_output(R):
    """R: [NBLK//4, 128, 4, OUT_FREE] device result -> [441, 64, 96] fp32."""
    dst, src = _gather_indices()
    O = np.zeros(D * D * H * W, dtype=np.float32)
    O[dst] = R.reshape(-1)[src].astype(np.float32)
    O *= np.float32(1.0 / C)
    return O.reshape(D * D, H, W)


def run_spmd(in_maps, **kwargs):
    from concourse import bass_utils

    return bass_utils.run_bass_kernel_spmd(
        _program(), in_maps, core_ids=list(range(N_CORES)), **kwargs
    )


def kernel(input1, input2):
    in_maps = make_in_maps(input1, input2)
    res = run_spmd(in_maps)
    return np.stack([extract_output(res.results[b]["out"]) for b in range(B)])
